# revision 4
# baseline (speedup 1.0000x reference)
"""GAT (2-layer, PyG-style) Trainium2 Bass kernel, 8-way edge/node-parallel.

Strategy (v2, dma_gather-based)
-------------------------------
* Nodes are degree-sorted into 128-node blocks dealt round-robin to 8 cores;
  each core owns ~N/8 destination nodes with balanced in-edge counts.
* Each core builds (replicated) a packed node table in ITS OWN node order
  (own nodes first): row g = 4 nodes x [h(64)|a_src(8)|a_dst(8)] f32 = 1280B,
  h/a_* from one PE matmul x @ [W1 | W1*att_src | W1*att_dst].  Packing 4
  nodes/row keeps dma_gather indices (= rowpos//4) within int16.
* Layer-1 edge pass: per owned block, k-chunks of the padded CSR slot grid are
  fetched with dma_gather (one 1280B row per slot); a host-uploaded one-hot
  (qoh) selects the right quarter on-chip.  Segment softmax is computed as a
  ratio of sums (exp without max subtraction; every node has a self-loop, pad
  slots hit a pad row whose a_src=-1e30 => weight exactly 0).
* Between layers each core's z rows [z(7)|za_src(1)] are AllGathered (3 MB);
  layer 2 repeats the same structure with 8-node-packed 256B z rows.
* log_softmax on-chip; host reassembles outputs via the inverse permutation.
"""
import os
import sys

import numpy as np

_REPO = "/opt/trn_rl_repo"
if os.path.isdir(_REPO) and _REPO not in sys.path:
    sys.path.insert(0, _REPO)

import concourse.bacc as bacc
import concourse.mybir as mybir
from concourse import library_config
from concourse.alu_op_type import AluOpType
from concourse.tile import TileContext

P = 128
F_IN = 128
HD = 64        # h width (= NH * DH)
NH = 8
DH = 8
RW = 80        # per-node table row: [h(64) | a_src(8) | a_dst(8)]
G1 = 4         # nodes packed per layer-1 table row (320 f32 = 1280B)
ZW = 8         # z row: [z(7) | za_src]
G2 = 8         # nodes packed per layer-2 row (64 f32 = 256B)
C_OUT = 7
CK = 16        # k-columns per layer-1 gather chunk (2048 slots/call)
CK2 = 32       # k-columns per layer-2 gather chunk (4096 slots/call)

f32 = mybir.dt.float32
i16 = mybir.dt.int16
Exp = mybir.ActivationFunctionType.Exp
Ln = mybir.ActivationFunctionType.Ln
X_AX = mybir.AxisListType.X


def _ceil_to(a, m):
    return -(-a // m) * m


def _mkap(ap, lst, extra_off=0):
    return type(ap)(ap.tensor, ap.offset + extra_off, lst)


def _wrap_idx(vals):
    """dma_gather index layout: value i at [i%16, i//16], replicated over the
    8 Q7 core partition-groups."""
    n = len(vals)
    w = vals.reshape(n // 16, 16).T
    return np.tile(w, (8, 1))  # [128, n//16]


def _chunks_of(K, ck):
    out = []
    while K > 0:
        out.append(min(ck, K))
        K -= out[-1]
    return out


def _host_prep(edge_index, N, n_cores):
    src = np.asarray(edge_index[0]).astype(np.int64)
    dst = np.asarray(edge_index[1]).astype(np.int64)
    loop = np.arange(N, dtype=np.int64)
    src = np.concatenate([src, loop])
    dst = np.concatenate([dst, loop])

    deg = np.bincount(dst, minlength=N).astype(np.int64)
    order = np.argsort(-deg, kind="stable").astype(np.int64)

    B_tot = _ceil_to(max(-(-N // P), 1), n_cores)
    if B_tot * P - N < 1:
        B_tot += n_cores          # guarantee >=1 pad node (the dummy target)
    N_pad = B_tot * P
    B_core = B_tot // n_cores
    NB = N_pad // P
    assert N_pad % G1 == 0

    nodes = np.concatenate([order, np.arange(N, N_pad, dtype=np.int64)])
    grid = nodes.reshape(B_tot, P)
    deg_pad = np.concatenate([deg, np.zeros(N_pad - N, np.int64)])

    eperm = np.argsort(dst, kind="stable")
    src_sorted = src[eperm]
    starts_full = np.concatenate([[0], np.cumsum(np.bincount(dst, minlength=N))])
    starts = np.zeros(N_pad, np.int64)
    starts[:N] = starts_full[:N]
    starts[N:] = starts_full[N]

    # K schedule per per-core block slot (same on every core -> same program)
    Ks = []
    for jb in range(B_core):
        m = 1
        for c in range(n_cores):
            m = max(m, int(deg_pad[grid[jb * n_cores + c]].max()))
        Ks.append(m)
    chunks1 = [_chunks_of(K, CK) for K in Ks]
    chunks2 = [_chunks_of(K, CK2) for K in Ks]
    S1 = sum(P * K for K in Ks)

    SH = B_core * P + P            # per-core z shard rows (+1 dummy block)
    agpos = np.zeros(N_pad, np.int64)
    own_all = []
    for c in range(n_cores):
        own = grid[c::n_cores].reshape(-1)
        own_all.append(own)
        agpos[own] = c * SH + np.arange(B_core * P)

    per_core = []
    for c in range(n_cores):
        own = grid[c::n_cores]                     # [B_core, P]
        own_flat = own.reshape(-1)
        rest = np.setdiff1d(np.arange(N_pad, dtype=np.int64), own_flat,
                            assume_unique=False)
        perm = np.concatenate([own_flat, rest])    # position -> node id
        rowpos = np.empty(N_pad, np.int64)
        rowpos[perm] = np.arange(N_pad)
        dummy_pos = rowpos[N]                      # node N is always a pad node

        idx1_parts, qoh1_parts = [], []
        idx2_parts, qoh2_parts = [], []
        for jb in range(B_core):
            nb = own[jb]
            dg = deg_pad[nb]
            st = starts[nb]
            K = Ks[jb]
            cols = np.arange(K)
            mask = cols[None, :] < dg[:, None]          # [P, K]
            pos = st[:, None] + np.where(mask, cols[None, :], 0)
            pos = np.minimum(pos, len(src_sorted) - 1)
            sv = src_sorted[pos]
            rp = np.where(mask, rowpos[sv], dummy_pos)  # [P, K] table positions
            ag = np.where(mask, agpos[sv], B_core * P)  # z-shard positions (c=0 dummy)
            ag = np.where(mask, ag, B_core * P)

            co = 0
            for ck in chunks1[jb]:
                blk = rp[:, co:co + ck]                 # [P, ck]
                g = (blk // G1).astype(np.int16)
                q = (blk % G1).astype(np.int64)
                idx1_parts.append(_wrap_idx(g.T.ravel()).ravel())
                oh = np.zeros((P, ck, G1), np.float32)
                np.put_along_axis(oh, q[:, :, None], 1.0, axis=2)
                qoh1_parts.append(oh.ravel())
                co += ck
            co = 0
            for ck in chunks2[jb]:
                blk = ag[:, co:co + ck]
                g = (blk // G2).astype(np.int16)
                q = (blk % G2).astype(np.int64)
                idx2_parts.append(_wrap_idx(g.T.ravel()).ravel())
                oh = np.zeros((P, ck, G2), np.float32)
                np.put_along_axis(oh, q[:, :, None], 1.0, axis=2)
                qoh2_parts.append(oh.ravel())
                co += ck

        marks = np.zeros((P, NB), np.float32)
        pm = perm.reshape(NB, P).T                  # node id at [p, tile]
        marks[pm >= N] = -1e30

        per_core.append({
            "idx1": np.concatenate(idx1_parts),
            "qoh1": np.concatenate(qoh1_parts),
            "idx2": np.concatenate(idx2_parts),
            "qoh2": np.concatenate(qoh2_parts),
            "marks": marks,
            "_perm": perm,
        })

    # offsets into idx/qoh arrays per (block, chunk)
    def offsets(chunks, g):
        o_i, o_q = [], []
        oi = oq = 0
        for jb in range(B_core):
            li, lq = [], []
            for ck in chunks[jb]:
                li.append(oi)
                lq.append(oq)
                oi += P * (P * ck // 16)   # wrapped idx blob: [128, P*ck/16]
                oq += P * ck * g
            o_i.append(li)
            o_q.append(lq)
        return o_i, o_q, oi, oq

    off1_i, off1_q, S1i, S1q = offsets(chunks1, G1)
    off2_i, off2_q, S2i, S2q = offsets(chunks2, G2)

    cfg = dict(n_cores=n_cores, B_core=B_core, NB=NB, N_pad=N_pad, SH=SH,
               Ks=Ks, chunks1=chunks1, chunks2=chunks2,
               off1_i=off1_i, off1_q=off1_q, off2_i=off2_i, off2_q=off2_q,
               S1i=S1i, S1q=S1q, S2i=S2i, S2q=S2q)
    return cfg, per_core, own_all


def _build(cfg):
    n_cores = cfg["n_cores"]
    B_core, NB, N_pad, SH = cfg["B_core"], cfg["NB"], cfg["N_pad"], cfg["SH"]
    Ks, chunks1, chunks2 = cfg["Ks"], cfg["chunks1"], cfg["chunks2"]
    off1_i, off1_q = cfg["off1_i"], cfg["off1_q"]
    off2_i, off2_q = cfg["off2_i"], cfg["off2_q"]
    T4 = N_pad // G1
    ZROWS = n_cores * SH // G2

    nc = bacc.Bacc("TRN2", num_devices=n_cores)

    xT_d = nc.dram_tensor("xT", [F_IN, N_pad], f32, kind="ExternalInput")
    w1_d = nc.dram_tensor("W1", [F_IN, HD], f32, kind="ExternalInput")
    aS_d = nc.dram_tensor("attS", [F_IN, HD], f32, kind="ExternalInput")
    aD_d = nc.dram_tensor("attD", [F_IN, HD], f32, kind="ExternalInput")
    b1_d = nc.dram_tensor("b1b", [P, HD], f32, kind="ExternalInput")
    w2_d = nc.dram_tensor("W2", [HD, C_OUT], f32, kind="ExternalInput")
    aS2_d = nc.dram_tensor("attS2", [HD, C_OUT], f32, kind="ExternalInput")
    aD2_d = nc.dram_tensor("attD2", [HD, C_OUT], f32, kind="ExternalInput")
    b2_d = nc.dram_tensor("b2b", [P, C_OUT], f32, kind="ExternalInput")
    mk_d = nc.dram_tensor("marks", [P, NB], f32, kind="ExternalInput")
    id_d = nc.dram_tensor("ident", [P, P], f32, kind="ExternalInput")
    idx1_d = nc.dram_tensor("idx1", [cfg["S1i"]], i16, kind="ExternalInput")
    qoh1_d = nc.dram_tensor("qoh1", [cfg["S1q"]], f32, kind="ExternalInput")
    idx2_d = nc.dram_tensor("idx2", [cfg["S2i"]], i16, kind="ExternalInput")
    qoh2_d = nc.dram_tensor("qoh2", [cfg["S2q"]], f32, kind="ExternalInput")

    table_d = nc.dram_tensor("table", [T4, G1 * RW], f32)
    zsh_d = nc.dram_tensor("zsh", [SH, ZW], f32)
    zag_space = "Shared" if n_cores > 4 else "Local"
    zag_d = nc.dram_tensor("zag", [ZROWS, G2 * ZW], f32, addr_space=zag_space)

    embo_d = nc.dram_tensor("emb_o", [B_core * P, HD], f32, kind="ExternalOutput")
    logp_d = nc.dram_tensor("logp_o", [B_core * P, C_OUT], f32, kind="ExternalOutput")

    with TileContext(nc) as tc:
        with (
            tc.tile_pool(name="const", bufs=1) as cp,
            tc.tile_pool(name="io", bufs=3) as iop,
            tc.tile_pool(name="gat", bufs=2) as gp,
            tc.tile_pool(name="wk", bufs=3) as wk,
            tc.tile_pool(name="ps", bufs=2, space="PSUM") as ps,
        ):
            with tc.tile_critical():
                nc.gpsimd.load_library(library_config.mlp)

            # ---------------- setup consts ----------------
            W1_t = cp.tile([F_IN, HD], f32)
            nc.sync.dma_start(W1_t[:], w1_d[:, :])
            aS_t = cp.tile([F_IN, HD], f32)
            nc.sync.dma_start(aS_t[:], aS_d[:, :])
            aD_t = cp.tile([F_IN, HD], f32)
            nc.sync.dma_start(aD_t[:], aD_d[:, :])
            b1_t = cp.tile([P, HD], f32)
            nc.sync.dma_start(b1_t[:], b1_d[:, :])
            b2_t = cp.tile([P, C_OUT], f32)
            nc.sync.dma_start(b2_t[:], b2_d[:, :])
            mk_t = cp.tile([P, NB], f32)
            nc.sync.dma_start(mk_t[:], mk_d[:, :])
            ident = cp.tile([P, P], f32)
            nc.sync.dma_start(ident[:], id_d[:, :])

            Wx_t = cp.tile([F_IN, RW], f32)
            tmp0 = wk.tile([F_IN, HD], f32, tag="tmp0")
            p0 = tmp0[:].ap[0]
            tmp0_v = _mkap(tmp0[:], [p0, [DH, NH], [1, DH]])
            nc.vector.tensor_tensor(tmp0[:], W1_t[:], aS_t[:], AluOpType.mult)
            nc.vector.tensor_reduce(Wx_t[:, 64:72], tmp0_v, X_AX, AluOpType.add)
            nc.vector.tensor_tensor(tmp0[:], W1_t[:], aD_t[:], AluOpType.mult)
            nc.vector.tensor_reduce(Wx_t[:, 72:80], tmp0_v, X_AX, AluOpType.add)
            nc.vector.tensor_copy(Wx_t[:, 0:64], W1_t[:])

            W2_t = cp.tile([HD, C_OUT], f32)
            nc.sync.dma_start(W2_t[:], w2_d[:, :])
            aS2_t = cp.tile([HD, C_OUT], f32)
            nc.sync.dma_start(aS2_t[:], aS2_d[:, :])
            aD2_t = cp.tile([HD, C_OUT], f32)
            nc.sync.dma_start(aD2_t[:], aD2_d[:, :])
            W2e_t = cp.tile([HD, 9], f32)
            tmp2 = wk.tile([HD, C_OUT], f32, tag="tmp2")
            nc.vector.tensor_copy(W2e_t[:, 0:C_OUT], W2_t[:])
            nc.vector.tensor_tensor(tmp2[:], W2_t[:], aS2_t[:], AluOpType.mult)
            nc.vector.tensor_reduce(W2e_t[:, 7:8], tmp2[:], X_AX, AluOpType.add)
            nc.vector.tensor_tensor(tmp2[:], W2_t[:], aD2_t[:], AluOpType.mult)
            nc.vector.tensor_reduce(W2e_t[:, 8:9], tmp2[:], X_AX, AluOpType.add)

            adst_sb = cp.tile([P, B_core, NH], f32)   # own-node a_dst, block order
            zad_t = cp.tile([P, B_core], f32)         # own-node za_dst

            zdum = wk.tile([P, ZW], f32, tag="zdum")
            nc.vector.memset(zdum[:], 0.0)
            nc.vector.memset(zdum[:, 7:8], -1e30)
            nc.sync.dma_start(zsh_d[B_core * P:B_core * P + P, :], zdum[:])

            # ---------------- P0: build packed node table ----------------
            for i in range(NB):
                xt = iop.tile([P, P], f32, tag="xt")
                nc.sync.dma_start(
                    xt[:], _mkap(xT_d[0:1, 0:1], [[N_pad, P], [1, P]], i * P))
                hp = ps.tile([P, RW], f32, tag="hp")
                nc.tensor.matmul(hp[:], lhsT=xt[:], rhs=Wx_t[:], start=True, stop=True)
                row = iop.tile([P, RW], f32, tag="row")
                nc.vector.tensor_copy(row[:, 0:64], hp[:, 0:64])
                nc.vector.tensor_scalar(row[:, 64:80], hp[:, 64:80],
                                        mk_t[:, i:i + 1], None, AluOpType.add)
                if i < B_core:
                    nc.vector.tensor_copy(adst_sb[:, i, :], row[:, 72:80])
                nc.sync.dma_start(
                    _mkap(table_d[0:1, 0:1],
                          [[G1 * RW, P // G1], [RW, G1], [1, RW]],
                          i * P * RW),
                    row[:])

            # ---------------- P1: layer-1 edge pass ----------------
            for jb in range(B_core):
                na = wk.tile([P, HD], f32, tag="na")
                nc.vector.memset(na[:], 0.0)
                da = wk.tile([P, NH], f32, tag="da")
                nc.vector.memset(da[:], 0.0)
                ad = adst_sb[:, jb, :]
                ad_ap = ad.ap
                for ci, ck in enumerate(chunks1[jb]):
                    n_idx = P * ck
                    ix = iop.tile([P, P * CK // 16], i16, tag="ix")
                    nc.sync.dma_start(
                        ix[:, 0:n_idx // 16],
                        _mkap(idx1_d[0:1], [[n_idx // 16, P], [1, n_idx // 16]],
                              off1_i[jb][ci]))
                    qh = iop.tile([P, CK, G1], f32, tag="qh")
                    nc.sync.dma_start(
                        qh[:, 0:ck, :],
                        _mkap(qoh1_d[0:1], [[ck * G1, P], [1, ck * G1]],
                              off1_q[jb][ci]))
                    G = gp.tile([P, CK, G1 * RW], f32, tag="G")
                    nc.gpsimd.dma_gather(
                        out_ap=G[:, 0:ck, :], in_ap=table_d[:, :],
                        idxs_ap=ix[:, 0:n_idx // 16],
                        num_idxs=n_idx, num_idxs_reg=n_idx,
                        elem_size=G1 * RW, single_packet=False)
                    # select quarter: hsel[p,k,:] = sum_q qoh * G[p,k,q,:]
                    T = gp.tile([P, CK, G1, RW], f32, tag="T")
                    Tp = T[:].ap[0]
                    T_v = _mkap(T[:], [Tp, [G1 * RW, ck], [RW, G1], [1, RW]])
                    G_v = _mkap(G[:], [G[:].ap[0], [G1 * RW, ck], [RW, G1], [1, RW]])
                    qh_v = _mkap(qh[:], [qh[:].ap[0], [G1, ck], [1, G1], [0, RW]])
                    nc.vector.tensor_tensor(T_v, G_v, qh_v, AluOpType.mult)
                    hs = gp.tile([P, CK, RW], f32, tag="hs")
                    hs_p = hs[:].ap[0]
                    T_r = _mkap(T[:], [Tp, [G1 * RW, ck], [1, RW], [RW, G1]])
                    nc.vector.tensor_reduce(
                        _mkap(hs[:], [hs_p, [RW, ck], [1, RW]]),
                        T_r, X_AX, AluOpType.add)
                    # e = a_src + a_dst ; lrelu ; exp
                    e = wk.tile([P, CK, NH], f32, tag="e")
                    ad_b = _mkap(ad, [ad_ap[0], [0, ck], ad_ap[1]])
                    nc.vector.tensor_tensor(
                        e[:, 0:ck, :],
                        _mkap(hs[:], [hs_p, [RW, ck], [1, NH]], 64),
                        ad_b, AluOpType.add)
                    nc.vector.scalar_tensor_tensor(e[:, 0:ck, :], e[:, 0:ck, :], 0.2,
                                                   e[:, 0:ck, :], AluOpType.mult,
                                                   AluOpType.max)
                    w = wk.tile([P, CK, NH], f32, tag="w")
                    nc.scalar.activation(w[:, 0:ck, :], e[:, 0:ck, :], Exp)
                    wp = w[:].ap[0]
                    dt_ = wk.tile([P, NH], f32, tag="dt")
                    w_perm = _mkap(w[:], [wp, [1, NH], [NH, ck]])
                    nc.vector.tensor_reduce(dt_[:], w_perm, X_AX, AluOpType.add)
                    nc.vector.tensor_add(da[:], da[:], dt_[:])
                    m = gp.tile([P, CK, HD], f32, tag="m")
                    mp = m[:].ap[0]
                    m_v = _mkap(m[:], [mp, [HD, ck], [DH, NH], [1, DH]])
                    h_v = _mkap(hs[:], [hs_p, [RW, ck], [DH, NH], [1, DH]])
                    w_v = _mkap(w[:], [wp, [NH, ck], [1, NH], [0, DH]])
                    nc.vector.tensor_tensor(m_v, h_v, w_v, AluOpType.mult)
                    nt_ = wk.tile([P, HD], f32, tag="nt")
                    m_perm = _mkap(m[:], [mp, [1, HD], [HD, ck]])
                    nc.vector.tensor_reduce(nt_[:], m_perm, X_AX, AluOpType.add)
                    nc.vector.tensor_add(na[:], na[:], nt_[:])
                # epilogue: emb = num/den + b1 ; z rows
                nc.vector.tensor_scalar_add(da[:], da[:], 1e-16)
                rc = wk.tile([P, NH], f32, tag="rc")
                nc.vector.reciprocal(rc[:], da[:])
                emb = wk.tile([P, HD], f32, tag="emb")
                ep = emb[:].ap[0]
                emb_v = _mkap(emb[:], [ep, [DH, NH], [1, DH]])
                na_v = _mkap(na[:], [na[:].ap[0], [DH, NH], [1, DH]])
                rc_v = _mkap(rc[:], [rc[:].ap[0], [1, NH], [0, DH]])
                nc.vector.tensor_tensor(emb_v, na_v, rc_v, AluOpType.mult)
                nc.vector.tensor_add(emb[:], emb[:], b1_t[:])
                nc.sync.dma_start(embo_d[jb * P:(jb + 1) * P, :], emb[:])
                # elu -> z = elu(emb) @ [W2 | W2@attS2 | W2@attD2]
                t1 = wk.tile([P, HD], f32, tag="t1")
                nc.vector.tensor_scalar(t1[:], emb[:], 0.0, None, AluOpType.min)
                u = wk.tile([P, HD], f32, tag="u")
                nc.scalar.activation(u[:], t1[:], Exp)
                t2 = wk.tile([P, HD], f32, tag="t2")
                nc.vector.tensor_scalar(t2[:], emb[:], 0.0, None, AluOpType.max)
                el = wk.tile([P, HD], f32, tag="el")
                nc.vector.scalar_tensor_tensor(el[:], u[:], -1.0, t2[:],
                                               AluOpType.add, AluOpType.add)
                eT_p = ps.tile([HD, P], f32, tag="eTp")
                nc.tensor.transpose(eT_p[:], el[:], ident[:])
                eT = wk.tile([HD, P], f32, tag="eT")
                nc.vector.tensor_copy(eT[:], eT_p[:])
                zx = ps.tile([P, 9], f32, tag="zx")
                nc.tensor.matmul(zx[:], lhsT=eT[:], rhs=W2e_t[:], start=True, stop=True)
                z8 = wk.tile([P, ZW], f32, tag="z8")
                nc.vector.tensor_copy(z8[:], zx[:, 0:ZW])
                nc.vector.tensor_copy(zad_t[:, jb:jb + 1], zx[:, 8:9])
                nc.sync.dma_start(zsh_d[jb * P:(jb + 1) * P, :], z8[:])

            # ---------------- P2: exchange z rows ----------------
            nc.gpsimd.collective_compute(
                "AllGather", AluOpType.bypass,
                replica_groups=[list(range(n_cores))],
                ins=[zsh_d[:, :]], outs=[zag_d[:, :]])

            # ---------------- P3: layer-2 edge pass ----------------
            for jb in range(B_core):
                n2 = wk.tile([P, C_OUT], f32, tag="n2")
                nc.vector.memset(n2[:], 0.0)
                d2 = wk.tile([P, 1], f32, tag="d2")
                nc.vector.memset(d2[:], 0.0)
                za = zad_t[:, jb:jb + 1]
                for ci, ck in enumerate(chunks2[jb]):
                    n_idx = P * ck
                    ix2 = iop.tile([P, P * CK2 // 16], i16, tag="ix2")
                    nc.sync.dma_start(
                        ix2[:, 0:n_idx // 16],
                        _mkap(idx2_d[0:1], [[n_idx // 16, P], [1, n_idx // 16]],
                              off2_i[jb][ci]))
                    qh2 = iop.tile([P, CK2, G2], f32, tag="qh2")
                    nc.sync.dma_start(
                        qh2[:, 0:ck, :],
                        _mkap(qoh2_d[0:1], [[ck * G2, P], [1, ck * G2]],
                              off2_q[jb][ci]))
                    Gz = gp.tile([P, CK2, G2 * ZW], f32, tag="Gz")
                    nc.gpsimd.dma_gather(
                        out_ap=Gz[:, 0:ck, :], in_ap=zag_d[:, :],
                        idxs_ap=ix2[:, 0:n_idx // 16],
                        num_idxs=n_idx, num_idxs_reg=n_idx,
                        elem_size=G2 * ZW, single_packet=False)
                    T2 = gp.tile([P, CK2, G2, ZW], f32, tag="T2")
                    T2p = T2[:].ap[0]
                    T2_v = _mkap(T2[:], [T2p, [G2 * ZW, ck], [ZW, G2], [1, ZW]])
                    Gz_v = _mkap(Gz[:], [Gz[:].ap[0], [G2 * ZW, ck], [ZW, G2], [1, ZW]])
                    qh2_v = _mkap(qh2[:], [qh2[:].ap[0], [G2, ck], [1, G2], [0, ZW]])
                    nc.vector.tensor_tensor(T2_v, Gz_v, qh2_v, AluOpType.mult)
                    zs = gp.tile([P, CK2, ZW], f32, tag="zs")
                    zs_p = zs[:].ap[0]
                    T2_r = _mkap(T2[:], [T2p, [G2 * ZW, ck], [1, ZW], [ZW, G2]])
                    nc.vector.tensor_reduce(
                        _mkap(zs[:], [zs_p, [ZW, ck], [1, ZW]]),
                        T2_r, X_AX, AluOpType.add)
                    e2 = wk.tile([P, CK2], f32, tag="e2")
                    za_b = _mkap(za, [za.ap[0], [0, ck]])
                    nc.vector.tensor_tensor(
                        e2[:, 0:ck],
                        _mkap(zs[:], [zs_p, [ZW, ck]], 7),
                        za_b, AluOpType.add)
                    nc.vector.scalar_tensor_tensor(e2[:, 0:ck], e2[:, 0:ck], 0.2,
                                                   e2[:, 0:ck], AluOpType.mult,
                                                   AluOpType.max)
                    w2 = wk.tile([P, CK2], f32, tag="w2")
                    nc.scalar.activation(w2[:, 0:ck], e2[:, 0:ck], Exp)
                    d2t = wk.tile([P, 1], f32, tag="d2t")
                    nc.vector.tensor_reduce(d2t[:], w2[:, 0:ck], X_AX, AluOpType.add)
                    nc.vector.tensor_add(d2[:], d2[:], d2t[:])
                    m2 = gp.tile([P, CK2, C_OUT], f32, tag="m2")
                    m2p = m2[:].ap[0]
                    m2_v = _mkap(m2[:], [m2p, [C_OUT, ck], [1, C_OUT]])
                    zs_v = _mkap(zs[:], [zs_p, [ZW, ck], [1, C_OUT]])
                    w2_v = _mkap(w2[:], [w2[:].ap[0], [1, ck], [0, C_OUT]])
                    nc.vector.tensor_tensor(m2_v, zs_v, w2_v, AluOpType.mult)
                    n2t = wk.tile([P, C_OUT], f32, tag="n2t")
                    m2_perm = _mkap(m2[:], [m2p, [1, C_OUT], [C_OUT, ck]])
                    nc.vector.tensor_reduce(n2t[:], m2_perm, X_AX, AluOpType.add)
                    nc.vector.tensor_add(n2[:], n2[:], n2t[:])
                # epilogue: logits + log_softmax
                nc.vector.tensor_scalar_add(d2[:], d2[:], 1e-16)
                rc2 = wk.tile([P, 1], f32, tag="rc2")
                nc.vector.reciprocal(rc2[:], d2[:])
                lg = wk.tile([P, C_OUT], f32, tag="lg")
                nc.vector.tensor_scalar(lg[:], n2[:], rc2[:, 0:1], None,
                                        AluOpType.mult)
                nc.vector.tensor_add(lg[:], lg[:], b2_t[:])
                mx = wk.tile([P, 1], f32, tag="mx")
                nc.vector.tensor_reduce(mx[:], lg[:], X_AX, AluOpType.max)
                s_ = wk.tile([P, C_OUT], f32, tag="s_")
                nc.vector.tensor_scalar(s_[:], lg[:], mx[:, 0:1], None,
                                        AluOpType.subtract)
                pp = wk.tile([P, C_OUT], f32, tag="pp")
                nc.scalar.activation(pp[:], s_[:], Exp)
                sm = wk.tile([P, 1], f32, tag="sm")
                nc.vector.tensor_reduce(sm[:], pp[:], X_AX, AluOpType.add)
                lt = wk.tile([P, 1], f32, tag="lt")
                nc.scalar.activation(lt[:], sm[:], Ln)
                lo = wk.tile([P, C_OUT], f32, tag="lo")
                nc.vector.tensor_scalar(lo[:], s_[:], lt[:, 0:1], None,
                                        AluOpType.subtract)
                nc.sync.dma_start(logp_d[jb * P:(jb + 1) * P, :], lo[:])
    nc.compile()
    return nc


def _make_inputs(inputs, cfg, per_core):
    x = np.asarray(inputs["x"], np.float32)
    N = x.shape[0]
    N_pad = cfg["N_pad"]
    shared = {
        "W1": np.asarray(inputs["W1"], np.float32),
        "attS": np.tile(np.asarray(inputs["att_src1"], np.float32).reshape(1, HD),
                        (F_IN, 1)),
        "attD": np.tile(np.asarray(inputs["att_dst1"], np.float32).reshape(1, HD),
                        (F_IN, 1)),
        "b1b": np.tile(np.asarray(inputs["b1"], np.float32).reshape(1, HD), (P, 1)),
        "W2": np.asarray(inputs["W2"], np.float32),
        "attS2": np.tile(np.asarray(inputs["att_src2"], np.float32).reshape(1, C_OUT),
                         (HD, 1)),
        "attD2": np.tile(np.asarray(inputs["att_dst2"], np.float32).reshape(1, C_OUT),
                         (HD, 1)),
        "b2b": np.tile(np.asarray(inputs["b2"], np.float32).reshape(1, C_OUT), (P, 1)),
        "ident": np.eye(P, dtype=np.float32),
    }
    xp = np.zeros((N_pad, F_IN), np.float32)
    xp[:N] = x
    in_maps = []
    for c in range(cfg["n_cores"]):
        m = dict(shared)
        perm = per_core[c]["_perm"]
        m["xT"] = np.ascontiguousarray(xp[perm].T)
        for k in ("idx1", "qoh1", "idx2", "qoh2", "marks"):
            m[k] = per_core[c][k]
        in_maps.append(m)
    return in_maps


def _assemble(cfg, own_all, N, results):
    emb = np.zeros((N, HD), np.float32)
    logp = np.zeros((N, C_OUT), np.float32)
    for c, res in enumerate(results):
        own = own_all[c]
        valid = own < N
        emb[own[valid]] = res["emb_o"][valid]
        logp[own[valid]] = res["logp_o"][valid]
    return emb, logp


def run_gat(inputs, n_cores=8, sim=False, trace=False):
    N = np.asarray(inputs["x"]).shape[0]
    cfg, per_core, own_all = _host_prep(inputs["edge_index"], N, n_cores)
    nc = _build(cfg)
    in_maps = _make_inputs(inputs, cfg, per_core)
    perf = None
    if sim:
        from concourse.bass_interp import CoreSim, MultiCoreSim
        if n_cores == 1:
            sims = [CoreSim(nc, require_finite=False, require_nnan=False)]
            for name, arr in in_maps[0].items():
                sims[0].tensor(name)[:] = arr
            sims[0].simulate()
        else:
            mcs = MultiCoreSim(nc, num_cores=n_cores,
                               require_finite=False, require_nnan=False)
            sims = list(mcs.cores.values())
            for c, s in enumerate(sims):
                for name, arr in in_maps[c].items():
                    s.tensor(name)[:] = arr
            mcs.simulate()
        results = [{"emb_o": s.tensor("emb_o"), "logp_o": s.tensor("logp_o")}
                   for s in sims]
    else:
        from concourse.bass_utils import run_bass_kernel_spmd
        r = run_bass_kernel_spmd(nc, in_maps, core_ids=list(range(n_cores)),
                                 trace=trace)
        results = r.results
        perf = r
    emb, logp = _assemble(cfg, own_all, N, results)
    return emb, logp, perf


def kernel(**inputs):
    emb, logp, _ = run_gat(inputs, n_cores=8)
    return emb, logp


# revision 5
# speedup vs baseline: 1.0082x; 1.0082x over previous
"""GAT (2-layer, PyG-style) Trainium2 Bass kernel, 8-way edge/node-parallel.

Strategy (v2, dma_gather-based)
-------------------------------
* Nodes are degree-sorted into 128-node blocks dealt round-robin to 8 cores;
  each core owns ~N/8 destination nodes with balanced in-edge counts.
* Each core builds (replicated) a packed node table in ITS OWN node order
  (own nodes first): row g = 4 nodes x [h(64)|a_src(8)|a_dst(8)] f32 = 1280B,
  h/a_* from one PE matmul x @ [W1 | W1*att_src | W1*att_dst].  Packing 4
  nodes/row keeps dma_gather indices (= rowpos//4) within int16.
* Layer-1 edge pass: per owned block, k-chunks of the padded CSR slot grid are
  fetched with dma_gather (one 1280B row per slot); a host-uploaded one-hot
  (qoh) selects the right quarter on-chip.  Segment softmax is computed as a
  ratio of sums (exp without max subtraction; every node has a self-loop, pad
  slots hit a pad row whose a_src=-1e30 => weight exactly 0).
* Between layers each core's z rows [z(7)|za_src(1)] are AllGathered (3 MB);
  layer 2 repeats the same structure with 8-node-packed 256B z rows.
* log_softmax on-chip; host reassembles outputs via the inverse permutation.
"""
import os
import sys

import numpy as np

_REPO = "/opt/trn_rl_repo"
if os.path.isdir(_REPO) and _REPO not in sys.path:
    sys.path.insert(0, _REPO)

import concourse.bacc as bacc
import concourse.mybir as mybir
from concourse import library_config
from concourse.alu_op_type import AluOpType
from concourse.tile import TileContext

P = 128
F_IN = 128
HD = 64        # h width (= NH * DH)
NH = 8
DH = 8
RW = 80        # per-node table row: [h(64) | a_src(8) | a_dst(8)]
G1 = 4         # nodes packed per layer-1 table row (320 f32 = 1280B)
ZW = 8         # z row: [z(7) | za_src]
G2 = 8         # nodes packed per layer-2 row (64 f32 = 256B)
C_OUT = 7
CK = 16        # k-columns per layer-1 gather chunk (2048 slots/call)
CK2 = 32       # k-columns per layer-2 gather chunk (4096 slots/call)

f32 = mybir.dt.float32
i16 = mybir.dt.int16
Exp = mybir.ActivationFunctionType.Exp
Ln = mybir.ActivationFunctionType.Ln
X_AX = mybir.AxisListType.X


def _ceil_to(a, m):
    return -(-a // m) * m


def _mkap(ap, lst, extra_off=0):
    return type(ap)(ap.tensor, ap.offset + extra_off, lst)


def _wrap_idx(vals):
    """dma_gather index layout: value i at [i%16, i//16], replicated over the
    8 Q7 core partition-groups."""
    n = len(vals)
    w = vals.reshape(n // 16, 16).T
    return np.tile(w, (8, 1))  # [128, n//16]


def _chunks_of(K, ck):
    out = []
    while K > 0:
        out.append(min(ck, K))
        K -= out[-1]
    return out


def _host_prep(edge_index, N, n_cores):
    src = np.asarray(edge_index[0]).astype(np.int64)
    dst = np.asarray(edge_index[1]).astype(np.int64)
    loop = np.arange(N, dtype=np.int64)
    src = np.concatenate([src, loop])
    dst = np.concatenate([dst, loop])

    deg = np.bincount(dst, minlength=N).astype(np.int64)
    order = np.argsort(-deg, kind="stable").astype(np.int64)

    B_tot = _ceil_to(max(-(-N // P), 1), n_cores)
    if B_tot * P - N < 1:
        B_tot += n_cores          # guarantee >=1 pad node (the dummy target)
    N_pad = B_tot * P
    B_core = B_tot // n_cores
    NB = N_pad // P
    assert N_pad % G1 == 0

    nodes = np.concatenate([order, np.arange(N, N_pad, dtype=np.int64)])
    grid = nodes.reshape(B_tot, P)
    deg_pad = np.concatenate([deg, np.zeros(N_pad - N, np.int64)])

    eperm = np.argsort(dst, kind="stable")
    src_sorted = src[eperm]
    starts_full = np.concatenate([[0], np.cumsum(np.bincount(dst, minlength=N))])
    starts = np.zeros(N_pad, np.int64)
    starts[:N] = starts_full[:N]
    starts[N:] = starts_full[N]

    # K schedule per per-core block slot (same on every core -> same program)
    Ks = []
    for jb in range(B_core):
        m = 1
        for c in range(n_cores):
            m = max(m, int(deg_pad[grid[jb * n_cores + c]].max()))
        Ks.append(m)
    chunks1 = [_chunks_of(K, CK) for K in Ks]
    chunks2 = [_chunks_of(K, CK2) for K in Ks]
    S1 = sum(P * K for K in Ks)

    SH = B_core * P + P            # per-core z shard rows (+1 dummy block)
    agpos = np.zeros(N_pad, np.int64)
    own_all = []
    for c in range(n_cores):
        own = grid[c::n_cores].reshape(-1)
        own_all.append(own)
        agpos[own] = c * SH + np.arange(B_core * P)

    per_core = []
    for c in range(n_cores):
        own = grid[c::n_cores]                     # [B_core, P]
        own_flat = own.reshape(-1)
        rest = np.setdiff1d(np.arange(N_pad, dtype=np.int64), own_flat,
                            assume_unique=False)
        perm = np.concatenate([own_flat, rest])    # position -> node id
        rowpos = np.empty(N_pad, np.int64)
        rowpos[perm] = np.arange(N_pad)
        dummy_pos = rowpos[N]                      # node N is always a pad node

        idx1_parts, qoh1_parts = [], []
        idx2_parts, qoh2_parts = [], []
        for jb in range(B_core):
            nb = own[jb]
            dg = deg_pad[nb]
            st = starts[nb]
            K = Ks[jb]
            cols = np.arange(K)
            mask = cols[None, :] < dg[:, None]          # [P, K]
            pos = st[:, None] + np.where(mask, cols[None, :], 0)
            pos = np.minimum(pos, len(src_sorted) - 1)
            sv = src_sorted[pos]
            rp = np.where(mask, rowpos[sv], dummy_pos)  # [P, K] table positions
            ag = np.where(mask, agpos[sv], B_core * P)  # z-shard positions (c=0 dummy)
            ag = np.where(mask, ag, B_core * P)

            co = 0
            for ck in chunks1[jb]:
                blk = rp[:, co:co + ck]                 # [P, ck]
                g = (blk // G1).astype(np.int16)
                q = (blk % G1).astype(np.int64)
                idx1_parts.append(_wrap_idx(g.T.ravel()).ravel())
                oh = np.zeros((P, ck, G1), np.float32)
                np.put_along_axis(oh, q[:, :, None], 1.0, axis=2)
                qoh1_parts.append(oh.ravel())
                co += ck
            co = 0
            for ck in chunks2[jb]:
                blk = ag[:, co:co + ck]
                g = (blk // G2).astype(np.int16)
                q = (blk % G2).astype(np.int64)
                idx2_parts.append(_wrap_idx(g.T.ravel()).ravel())
                oh = np.zeros((P, ck, G2), np.float32)
                np.put_along_axis(oh, q[:, :, None], 1.0, axis=2)
                qoh2_parts.append(oh.ravel())
                co += ck

        marks = np.zeros((P, NB), np.float32)
        pm = perm.reshape(NB, P).T                  # node id at [p, tile]
        marks[pm >= N] = -1e30

        per_core.append({
            "idx1": np.concatenate(idx1_parts),
            "qoh1": np.concatenate(qoh1_parts),
            "idx2": np.concatenate(idx2_parts),
            "qoh2": np.concatenate(qoh2_parts),
            "marks": marks,
            "_perm": perm,
        })

    # offsets into idx/qoh arrays per (block, chunk)
    def offsets(chunks, g):
        o_i, o_q = [], []
        oi = oq = 0
        for jb in range(B_core):
            li, lq = [], []
            for ck in chunks[jb]:
                li.append(oi)
                lq.append(oq)
                oi += P * (P * ck // 16)   # wrapped idx blob: [128, P*ck/16]
                oq += P * ck * g
            o_i.append(li)
            o_q.append(lq)
        return o_i, o_q, oi, oq

    off1_i, off1_q, S1i, S1q = offsets(chunks1, G1)
    off2_i, off2_q, S2i, S2q = offsets(chunks2, G2)

    cfg = dict(n_cores=n_cores, B_core=B_core, NB=NB, N_pad=N_pad, SH=SH,
               Ks=Ks, chunks1=chunks1, chunks2=chunks2,
               off1_i=off1_i, off1_q=off1_q, off2_i=off2_i, off2_q=off2_q,
               S1i=S1i, S1q=S1q, S2i=S2i, S2q=S2q)
    return cfg, per_core, own_all


def _build(cfg):
    n_cores = cfg["n_cores"]
    B_core, NB, N_pad, SH = cfg["B_core"], cfg["NB"], cfg["N_pad"], cfg["SH"]
    Ks, chunks1, chunks2 = cfg["Ks"], cfg["chunks1"], cfg["chunks2"]
    off1_i, off1_q = cfg["off1_i"], cfg["off1_q"]
    off2_i, off2_q = cfg["off2_i"], cfg["off2_q"]
    T4 = N_pad // G1
    ZROWS = n_cores * SH // G2

    nc = bacc.Bacc("TRN2", num_devices=n_cores, num_swdge_queues=4)

    xT_d = nc.dram_tensor("xT", [F_IN, N_pad], f32, kind="ExternalInput")
    w1_d = nc.dram_tensor("W1", [F_IN, HD], f32, kind="ExternalInput")
    aS_d = nc.dram_tensor("attS", [F_IN, HD], f32, kind="ExternalInput")
    aD_d = nc.dram_tensor("attD", [F_IN, HD], f32, kind="ExternalInput")
    b1_d = nc.dram_tensor("b1b", [P, HD], f32, kind="ExternalInput")
    w2_d = nc.dram_tensor("W2", [HD, C_OUT], f32, kind="ExternalInput")
    aS2_d = nc.dram_tensor("attS2", [HD, C_OUT], f32, kind="ExternalInput")
    aD2_d = nc.dram_tensor("attD2", [HD, C_OUT], f32, kind="ExternalInput")
    b2_d = nc.dram_tensor("b2b", [P, C_OUT], f32, kind="ExternalInput")
    mk_d = nc.dram_tensor("marks", [P, NB], f32, kind="ExternalInput")
    id_d = nc.dram_tensor("ident", [P, P], f32, kind="ExternalInput")
    idx1_d = nc.dram_tensor("idx1", [cfg["S1i"]], i16, kind="ExternalInput")
    qoh1_d = nc.dram_tensor("qoh1", [cfg["S1q"]], f32, kind="ExternalInput")
    idx2_d = nc.dram_tensor("idx2", [cfg["S2i"]], i16, kind="ExternalInput")
    qoh2_d = nc.dram_tensor("qoh2", [cfg["S2q"]], f32, kind="ExternalInput")

    table_d = nc.dram_tensor("table", [T4, G1 * RW], f32)
    zsh_d = nc.dram_tensor("zsh", [SH, ZW], f32)
    zag_space = "Shared" if n_cores > 4 else "Local"
    zag_d = nc.dram_tensor("zag", [ZROWS, G2 * ZW], f32, addr_space=zag_space)

    embo_d = nc.dram_tensor("emb_o", [B_core * P, HD], f32, kind="ExternalOutput")
    logp_d = nc.dram_tensor("logp_o", [B_core * P, C_OUT], f32, kind="ExternalOutput")

    with TileContext(nc) as tc:
        with (
            tc.tile_pool(name="const", bufs=1) as cp,
            tc.tile_pool(name="io", bufs=3) as iop,
            tc.tile_pool(name="gat", bufs=2) as gp,
            tc.tile_pool(name="wk", bufs=3) as wk,
            tc.tile_pool(name="ps", bufs=2, space="PSUM") as ps,
        ):
            with tc.tile_critical():
                nc.gpsimd.load_library(library_config.mlp)
            qcnt = [0]

            # ---------------- setup consts ----------------
            W1_t = cp.tile([F_IN, HD], f32)
            nc.sync.dma_start(W1_t[:], w1_d[:, :])
            aS_t = cp.tile([F_IN, HD], f32)
            nc.sync.dma_start(aS_t[:], aS_d[:, :])
            aD_t = cp.tile([F_IN, HD], f32)
            nc.sync.dma_start(aD_t[:], aD_d[:, :])
            b1_t = cp.tile([P, HD], f32)
            nc.sync.dma_start(b1_t[:], b1_d[:, :])
            b2_t = cp.tile([P, C_OUT], f32)
            nc.sync.dma_start(b2_t[:], b2_d[:, :])
            mk_t = cp.tile([P, NB], f32)
            nc.sync.dma_start(mk_t[:], mk_d[:, :])
            ident = cp.tile([P, P], f32)
            nc.sync.dma_start(ident[:], id_d[:, :])

            Wx_t = cp.tile([F_IN, RW], f32)
            tmp0 = wk.tile([F_IN, HD], f32, tag="tmp0")
            p0 = tmp0[:].ap[0]
            tmp0_v = _mkap(tmp0[:], [p0, [DH, NH], [1, DH]])
            nc.vector.tensor_tensor(tmp0[:], W1_t[:], aS_t[:], AluOpType.mult)
            nc.vector.tensor_reduce(Wx_t[:, 64:72], tmp0_v, X_AX, AluOpType.add)
            nc.vector.tensor_tensor(tmp0[:], W1_t[:], aD_t[:], AluOpType.mult)
            nc.vector.tensor_reduce(Wx_t[:, 72:80], tmp0_v, X_AX, AluOpType.add)
            nc.vector.tensor_copy(Wx_t[:, 0:64], W1_t[:])

            W2_t = cp.tile([HD, C_OUT], f32)
            nc.sync.dma_start(W2_t[:], w2_d[:, :])
            aS2_t = cp.tile([HD, C_OUT], f32)
            nc.sync.dma_start(aS2_t[:], aS2_d[:, :])
            aD2_t = cp.tile([HD, C_OUT], f32)
            nc.sync.dma_start(aD2_t[:], aD2_d[:, :])
            W2e_t = cp.tile([HD, 9], f32)
            tmp2 = wk.tile([HD, C_OUT], f32, tag="tmp2")
            nc.vector.tensor_copy(W2e_t[:, 0:C_OUT], W2_t[:])
            nc.vector.tensor_tensor(tmp2[:], W2_t[:], aS2_t[:], AluOpType.mult)
            nc.vector.tensor_reduce(W2e_t[:, 7:8], tmp2[:], X_AX, AluOpType.add)
            nc.vector.tensor_tensor(tmp2[:], W2_t[:], aD2_t[:], AluOpType.mult)
            nc.vector.tensor_reduce(W2e_t[:, 8:9], tmp2[:], X_AX, AluOpType.add)

            adst_sb = cp.tile([P, B_core, NH], f32)   # own-node a_dst, block order
            zad_t = cp.tile([P, B_core], f32)         # own-node za_dst

            zdum = wk.tile([P, ZW], f32, tag="zdum")
            nc.vector.memset(zdum[:], 0.0)
            nc.vector.memset(zdum[:, 7:8], -1e30)
            nc.sync.dma_start(zsh_d[B_core * P:B_core * P + P, :], zdum[:])

            # ---------------- P0: build packed node table ----------------
            for i in range(NB):
                xt = iop.tile([P, P], f32, tag="xt")
                nc.sync.dma_start(
                    xt[:], _mkap(xT_d[0:1, 0:1], [[N_pad, P], [1, P]], i * P))
                hp = ps.tile([P, RW], f32, tag="hp")
                nc.tensor.matmul(hp[:], lhsT=xt[:], rhs=Wx_t[:], start=True, stop=True)
                row = iop.tile([P, RW], f32, tag="row")
                nc.vector.tensor_copy(row[:, 0:64], hp[:, 0:64])
                nc.vector.tensor_scalar(row[:, 64:80], hp[:, 64:80],
                                        mk_t[:, i:i + 1], None, AluOpType.add)
                if i < B_core:
                    nc.vector.tensor_copy(adst_sb[:, i, :], row[:, 72:80])
                nc.sync.dma_start(
                    _mkap(table_d[0:1, 0:1],
                          [[G1 * RW, P // G1], [RW, G1], [1, RW]],
                          i * P * RW),
                    row[:])

            # ---------------- P1: layer-1 edge pass ----------------
            for jb in range(B_core):
                na = wk.tile([P, HD], f32, tag="na")
                nc.vector.memset(na[:], 0.0)
                da = wk.tile([P, NH], f32, tag="da")
                nc.vector.memset(da[:], 0.0)
                ad = adst_sb[:, jb, :]
                ad_ap = ad.ap
                for ci, ck in enumerate(chunks1[jb]):
                    n_idx = P * ck
                    ix = iop.tile([P, P * CK // 16], i16, tag="ix")
                    nc.sync.dma_start(
                        ix[:, 0:n_idx // 16],
                        _mkap(idx1_d[0:1], [[n_idx // 16, P], [1, n_idx // 16]],
                              off1_i[jb][ci]))
                    qh = iop.tile([P, CK, G1], f32, tag="qh")
                    nc.sync.dma_start(
                        qh[:, 0:ck, :],
                        _mkap(qoh1_d[0:1], [[ck * G1, P], [1, ck * G1]],
                              off1_q[jb][ci]))
                    G = gp.tile([P, CK, G1 * RW], f32, tag="G")
                    nc.gpsimd.dma_gather(
                        out_ap=G[:, 0:ck, :], in_ap=table_d[:, :],
                        idxs_ap=ix[:, 0:n_idx // 16],
                        num_idxs=n_idx, num_idxs_reg=n_idx,
                        elem_size=G1 * RW, single_packet=False,
                        queue_num=qcnt[0] % 4)
                    qcnt[0] += 1
                    # select quarter: hsel[p,k,:] = sum_q qoh * G[p,k,q,:]
                    T = gp.tile([P, CK, G1, RW], f32, tag="T")
                    Tp = T[:].ap[0]
                    T_v = _mkap(T[:], [Tp, [G1 * RW, ck], [RW, G1], [1, RW]])
                    G_v = _mkap(G[:], [G[:].ap[0], [G1 * RW, ck], [RW, G1], [1, RW]])
                    qh_v = _mkap(qh[:], [qh[:].ap[0], [G1, ck], [1, G1], [0, RW]])
                    nc.vector.tensor_tensor(T_v, G_v, qh_v, AluOpType.mult)
                    hs = gp.tile([P, CK, RW], f32, tag="hs")
                    hs_p = hs[:].ap[0]
                    T_r = _mkap(T[:], [Tp, [G1 * RW, ck], [1, RW], [RW, G1]])
                    nc.vector.tensor_reduce(
                        _mkap(hs[:], [hs_p, [RW, ck], [1, RW]]),
                        T_r, X_AX, AluOpType.add)
                    # e = a_src + a_dst ; lrelu ; exp
                    e = wk.tile([P, CK, NH], f32, tag="e")
                    ad_b = _mkap(ad, [ad_ap[0], [0, ck], ad_ap[1]])
                    nc.vector.tensor_tensor(
                        e[:, 0:ck, :],
                        _mkap(hs[:], [hs_p, [RW, ck], [1, NH]], 64),
                        ad_b, AluOpType.add)
                    nc.vector.scalar_tensor_tensor(e[:, 0:ck, :], e[:, 0:ck, :], 0.2,
                                                   e[:, 0:ck, :], AluOpType.mult,
                                                   AluOpType.max)
                    w = wk.tile([P, CK, NH], f32, tag="w")
                    nc.scalar.activation(w[:, 0:ck, :], e[:, 0:ck, :], Exp)
                    wp = w[:].ap[0]
                    dt_ = wk.tile([P, NH], f32, tag="dt")
                    w_perm = _mkap(w[:], [wp, [1, NH], [NH, ck]])
                    nc.vector.tensor_reduce(dt_[:], w_perm, X_AX, AluOpType.add)
                    nc.vector.tensor_add(da[:], da[:], dt_[:])
                    m = gp.tile([P, CK, HD], f32, tag="m")
                    mp = m[:].ap[0]
                    m_v = _mkap(m[:], [mp, [HD, ck], [DH, NH], [1, DH]])
                    h_v = _mkap(hs[:], [hs_p, [RW, ck], [DH, NH], [1, DH]])
                    w_v = _mkap(w[:], [wp, [NH, ck], [1, NH], [0, DH]])
                    nc.vector.tensor_tensor(m_v, h_v, w_v, AluOpType.mult)
                    nt_ = wk.tile([P, HD], f32, tag="nt")
                    m_perm = _mkap(m[:], [mp, [1, HD], [HD, ck]])
                    nc.vector.tensor_reduce(nt_[:], m_perm, X_AX, AluOpType.add)
                    nc.vector.tensor_add(na[:], na[:], nt_[:])
                # epilogue: emb = num/den + b1 ; z rows
                nc.vector.tensor_scalar_add(da[:], da[:], 1e-16)
                rc = wk.tile([P, NH], f32, tag="rc")
                nc.vector.reciprocal(rc[:], da[:])
                emb = wk.tile([P, HD], f32, tag="emb")
                ep = emb[:].ap[0]
                emb_v = _mkap(emb[:], [ep, [DH, NH], [1, DH]])
                na_v = _mkap(na[:], [na[:].ap[0], [DH, NH], [1, DH]])
                rc_v = _mkap(rc[:], [rc[:].ap[0], [1, NH], [0, DH]])
                nc.vector.tensor_tensor(emb_v, na_v, rc_v, AluOpType.mult)
                nc.vector.tensor_add(emb[:], emb[:], b1_t[:])
                nc.sync.dma_start(embo_d[jb * P:(jb + 1) * P, :], emb[:])
                # elu -> z = elu(emb) @ [W2 | W2@attS2 | W2@attD2]
                t1 = wk.tile([P, HD], f32, tag="t1")
                nc.vector.tensor_scalar(t1[:], emb[:], 0.0, None, AluOpType.min)
                u = wk.tile([P, HD], f32, tag="u")
                nc.scalar.activation(u[:], t1[:], Exp)
                t2 = wk.tile([P, HD], f32, tag="t2")
                nc.vector.tensor_scalar(t2[:], emb[:], 0.0, None, AluOpType.max)
                el = wk.tile([P, HD], f32, tag="el")
                nc.vector.scalar_tensor_tensor(el[:], u[:], -1.0, t2[:],
                                               AluOpType.add, AluOpType.add)
                eT_p = ps.tile([HD, P], f32, tag="eTp")
                nc.tensor.transpose(eT_p[:], el[:], ident[:])
                eT = wk.tile([HD, P], f32, tag="eT")
                nc.vector.tensor_copy(eT[:], eT_p[:])
                zx = ps.tile([P, 9], f32, tag="zx")
                nc.tensor.matmul(zx[:], lhsT=eT[:], rhs=W2e_t[:], start=True, stop=True)
                z8 = wk.tile([P, ZW], f32, tag="z8")
                nc.vector.tensor_copy(z8[:], zx[:, 0:ZW])
                nc.vector.tensor_copy(zad_t[:, jb:jb + 1], zx[:, 8:9])
                nc.sync.dma_start(zsh_d[jb * P:(jb + 1) * P, :], z8[:])

            # ---------------- P2: exchange z rows ----------------
            nc.gpsimd.collective_compute(
                "AllGather", AluOpType.bypass,
                replica_groups=[list(range(n_cores))],
                ins=[zsh_d[:, :]], outs=[zag_d[:, :]])

            # ---------------- P3: layer-2 edge pass ----------------
            for jb in range(B_core):
                n2 = wk.tile([P, C_OUT], f32, tag="n2")
                nc.vector.memset(n2[:], 0.0)
                d2 = wk.tile([P, 1], f32, tag="d2")
                nc.vector.memset(d2[:], 0.0)
                za = zad_t[:, jb:jb + 1]
                for ci, ck in enumerate(chunks2[jb]):
                    n_idx = P * ck
                    ix2 = iop.tile([P, P * CK2 // 16], i16, tag="ix2")
                    nc.sync.dma_start(
                        ix2[:, 0:n_idx // 16],
                        _mkap(idx2_d[0:1], [[n_idx // 16, P], [1, n_idx // 16]],
                              off2_i[jb][ci]))
                    qh2 = iop.tile([P, CK2, G2], f32, tag="qh2")
                    nc.sync.dma_start(
                        qh2[:, 0:ck, :],
                        _mkap(qoh2_d[0:1], [[ck * G2, P], [1, ck * G2]],
                              off2_q[jb][ci]))
                    Gz = gp.tile([P, CK2, G2 * ZW], f32, tag="Gz")
                    nc.gpsimd.dma_gather(
                        out_ap=Gz[:, 0:ck, :], in_ap=zag_d[:, :],
                        idxs_ap=ix2[:, 0:n_idx // 16],
                        num_idxs=n_idx, num_idxs_reg=n_idx,
                        elem_size=G2 * ZW, single_packet=False,
                        queue_num=qcnt[0] % 4)
                    qcnt[0] += 1
                    T2 = gp.tile([P, CK2, G2, ZW], f32, tag="T2")
                    T2p = T2[:].ap[0]
                    T2_v = _mkap(T2[:], [T2p, [G2 * ZW, ck], [ZW, G2], [1, ZW]])
                    Gz_v = _mkap(Gz[:], [Gz[:].ap[0], [G2 * ZW, ck], [ZW, G2], [1, ZW]])
                    qh2_v = _mkap(qh2[:], [qh2[:].ap[0], [G2, ck], [1, G2], [0, ZW]])
                    nc.vector.tensor_tensor(T2_v, Gz_v, qh2_v, AluOpType.mult)
                    zs = gp.tile([P, CK2, ZW], f32, tag="zs")
                    zs_p = zs[:].ap[0]
                    T2_r = _mkap(T2[:], [T2p, [G2 * ZW, ck], [1, ZW], [ZW, G2]])
                    nc.vector.tensor_reduce(
                        _mkap(zs[:], [zs_p, [ZW, ck], [1, ZW]]),
                        T2_r, X_AX, AluOpType.add)
                    e2 = wk.tile([P, CK2], f32, tag="e2")
                    za_b = _mkap(za, [za.ap[0], [0, ck]])
                    nc.vector.tensor_tensor(
                        e2[:, 0:ck],
                        _mkap(zs[:], [zs_p, [ZW, ck]], 7),
                        za_b, AluOpType.add)
                    nc.vector.scalar_tensor_tensor(e2[:, 0:ck], e2[:, 0:ck], 0.2,
                                                   e2[:, 0:ck], AluOpType.mult,
                                                   AluOpType.max)
                    w2 = wk.tile([P, CK2], f32, tag="w2")
                    nc.scalar.activation(w2[:, 0:ck], e2[:, 0:ck], Exp)
                    d2t = wk.tile([P, 1], f32, tag="d2t")
                    nc.vector.tensor_reduce(d2t[:], w2[:, 0:ck], X_AX, AluOpType.add)
                    nc.vector.tensor_add(d2[:], d2[:], d2t[:])
                    m2 = gp.tile([P, CK2, C_OUT], f32, tag="m2")
                    m2p = m2[:].ap[0]
                    m2_v = _mkap(m2[:], [m2p, [C_OUT, ck], [1, C_OUT]])
                    zs_v = _mkap(zs[:], [zs_p, [ZW, ck], [1, C_OUT]])
                    w2_v = _mkap(w2[:], [w2[:].ap[0], [1, ck], [0, C_OUT]])
                    nc.vector.tensor_tensor(m2_v, zs_v, w2_v, AluOpType.mult)
                    n2t = wk.tile([P, C_OUT], f32, tag="n2t")
                    m2_perm = _mkap(m2[:], [m2p, [1, C_OUT], [C_OUT, ck]])
                    nc.vector.tensor_reduce(n2t[:], m2_perm, X_AX, AluOpType.add)
                    nc.vector.tensor_add(n2[:], n2[:], n2t[:])
                # epilogue: logits + log_softmax
                nc.vector.tensor_scalar_add(d2[:], d2[:], 1e-16)
                rc2 = wk.tile([P, 1], f32, tag="rc2")
                nc.vector.reciprocal(rc2[:], d2[:])
                lg = wk.tile([P, C_OUT], f32, tag="lg")
                nc.vector.tensor_scalar(lg[:], n2[:], rc2[:, 0:1], None,
                                        AluOpType.mult)
                nc.vector.tensor_add(lg[:], lg[:], b2_t[:])
                mx = wk.tile([P, 1], f32, tag="mx")
                nc.vector.tensor_reduce(mx[:], lg[:], X_AX, AluOpType.max)
                s_ = wk.tile([P, C_OUT], f32, tag="s_")
                nc.vector.tensor_scalar(s_[:], lg[:], mx[:, 0:1], None,
                                        AluOpType.subtract)
                pp = wk.tile([P, C_OUT], f32, tag="pp")
                nc.scalar.activation(pp[:], s_[:], Exp)
                sm = wk.tile([P, 1], f32, tag="sm")
                nc.vector.tensor_reduce(sm[:], pp[:], X_AX, AluOpType.add)
                lt = wk.tile([P, 1], f32, tag="lt")
                nc.scalar.activation(lt[:], sm[:], Ln)
                lo = wk.tile([P, C_OUT], f32, tag="lo")
                nc.vector.tensor_scalar(lo[:], s_[:], lt[:, 0:1], None,
                                        AluOpType.subtract)
                nc.sync.dma_start(logp_d[jb * P:(jb + 1) * P, :], lo[:])
    nc.compile()
    return nc


def _make_inputs(inputs, cfg, per_core):
    x = np.asarray(inputs["x"], np.float32)
    N = x.shape[0]
    N_pad = cfg["N_pad"]
    shared = {
        "W1": np.asarray(inputs["W1"], np.float32),
        "attS": np.tile(np.asarray(inputs["att_src1"], np.float32).reshape(1, HD),
                        (F_IN, 1)),
        "attD": np.tile(np.asarray(inputs["att_dst1"], np.float32).reshape(1, HD),
                        (F_IN, 1)),
        "b1b": np.tile(np.asarray(inputs["b1"], np.float32).reshape(1, HD), (P, 1)),
        "W2": np.asarray(inputs["W2"], np.float32),
        "attS2": np.tile(np.asarray(inputs["att_src2"], np.float32).reshape(1, C_OUT),
                         (HD, 1)),
        "attD2": np.tile(np.asarray(inputs["att_dst2"], np.float32).reshape(1, C_OUT),
                         (HD, 1)),
        "b2b": np.tile(np.asarray(inputs["b2"], np.float32).reshape(1, C_OUT), (P, 1)),
        "ident": np.eye(P, dtype=np.float32),
    }
    xp = np.zeros((N_pad, F_IN), np.float32)
    xp[:N] = x
    in_maps = []
    for c in range(cfg["n_cores"]):
        m = dict(shared)
        perm = per_core[c]["_perm"]
        m["xT"] = np.ascontiguousarray(xp[perm].T)
        for k in ("idx1", "qoh1", "idx2", "qoh2", "marks"):
            m[k] = per_core[c][k]
        in_maps.append(m)
    return in_maps


def _assemble(cfg, own_all, N, results):
    emb = np.zeros((N, HD), np.float32)
    logp = np.zeros((N, C_OUT), np.float32)
    for c, res in enumerate(results):
        own = own_all[c]
        valid = own < N
        emb[own[valid]] = res["emb_o"][valid]
        logp[own[valid]] = res["logp_o"][valid]
    return emb, logp


def run_gat(inputs, n_cores=8, sim=False, trace=False):
    N = np.asarray(inputs["x"]).shape[0]
    cfg, per_core, own_all = _host_prep(inputs["edge_index"], N, n_cores)
    nc = _build(cfg)
    in_maps = _make_inputs(inputs, cfg, per_core)
    perf = None
    if sim:
        from concourse.bass_interp import CoreSim, MultiCoreSim
        if n_cores == 1:
            sims = [CoreSim(nc, require_finite=False, require_nnan=False)]
            for name, arr in in_maps[0].items():
                sims[0].tensor(name)[:] = arr
            sims[0].simulate()
        else:
            mcs = MultiCoreSim(nc, num_cores=n_cores,
                               require_finite=False, require_nnan=False)
            sims = list(mcs.cores.values())
            for c, s in enumerate(sims):
                for name, arr in in_maps[c].items():
                    s.tensor(name)[:] = arr
            mcs.simulate()
        results = [{"emb_o": s.tensor("emb_o"), "logp_o": s.tensor("logp_o")}
                   for s in sims]
    else:
        from concourse.bass_utils import run_bass_kernel_spmd
        r = run_bass_kernel_spmd(nc, in_maps, core_ids=list(range(n_cores)),
                                 trace=trace)
        results = r.results
        perf = r
    emb, logp = _assemble(cfg, own_all, N, results)
    return emb, logp, perf


def kernel(**inputs):
    emb, logp, _ = run_gat(inputs, n_cores=8)
    return emb, logp


# revision 7
# speedup vs baseline: 1.2005x; 1.1907x over previous
"""GAT (2-layer, PyG-style) Trainium2 Bass kernel, 8-way edge/node-parallel.

Strategy (v2, dma_gather-based)
-------------------------------
* Nodes are degree-sorted into 128-node blocks dealt round-robin to 8 cores;
  each core owns ~N/8 destination nodes with balanced in-edge counts.
* Each core builds (replicated) a packed node table in ITS OWN node order
  (own nodes first): row g = 4 nodes x [h(64)|a_src(8)|a_dst(8)] f32 = 1280B,
  h/a_* from one PE matmul x @ [W1 | W1*att_src | W1*att_dst].  Packing 4
  nodes/row keeps dma_gather indices (= rowpos//4) within int16.
* Layer-1 edge pass: per owned block, k-chunks of the padded CSR slot grid are
  fetched with dma_gather (one 1280B row per slot); a host-uploaded one-hot
  (qoh) selects the right quarter on-chip.  Segment softmax is computed as a
  ratio of sums (exp without max subtraction; every node has a self-loop, pad
  slots hit a pad row whose a_src=-1e30 => weight exactly 0).
* Between layers each core's z rows [z(7)|za_src(1)] are AllGathered (3 MB);
  layer 2 repeats the same structure with 8-node-packed 256B z rows.
* log_softmax on-chip; host reassembles outputs via the inverse permutation.
"""
import os
import sys

import numpy as np

_REPO = "/opt/trn_rl_repo"
if os.path.isdir(_REPO) and _REPO not in sys.path:
    sys.path.insert(0, _REPO)

import concourse.bacc as bacc
import concourse.mybir as mybir
from concourse import library_config
from concourse.alu_op_type import AluOpType
from concourse.tile import TileContext

P = 128
F_IN = 128
HD = 64        # h width (= NH * DH)
NH = 8
DH = 8
RW = 80        # per-node table row: [h(64) | a_src(8) | a_dst(8)]
G1 = 4         # nodes packed per layer-1 table row (320 f32 = 1280B)
ZW = 8         # z row: [z(7) | za_src]
G2 = 8         # nodes packed per layer-2 row (64 f32 = 256B)
C_OUT = 7
CK = 8         # k-columns per layer-1 gather chunk (1024 slots/call)
CK2 = 16       # k-columns per layer-2 gather chunk (2048 slots/call)

f32 = mybir.dt.float32
i16 = mybir.dt.int16
Exp = mybir.ActivationFunctionType.Exp
Ln = mybir.ActivationFunctionType.Ln
X_AX = mybir.AxisListType.X


def _ceil_to(a, m):
    return -(-a // m) * m


def _mkap(ap, lst, extra_off=0):
    return type(ap)(ap.tensor, ap.offset + extra_off, lst)


def _wrap_idx(vals):
    """dma_gather index layout: value i at [i%16, i//16], replicated over the
    8 Q7 core partition-groups."""
    n = len(vals)
    w = vals.reshape(n // 16, 16).T
    return np.tile(w, (8, 1))  # [128, n//16]


def _chunks_of(K, ck):
    out = []
    while K > 0:
        out.append(min(ck, K))
        K -= out[-1]
    return out


def _host_prep(edge_index, N, n_cores):
    src = np.asarray(edge_index[0]).astype(np.int64)
    dst = np.asarray(edge_index[1]).astype(np.int64)
    loop = np.arange(N, dtype=np.int64)
    src = np.concatenate([src, loop])
    dst = np.concatenate([dst, loop])

    deg = np.bincount(dst, minlength=N).astype(np.int64)
    order = np.argsort(-deg, kind="stable").astype(np.int64)

    B_tot = _ceil_to(max(-(-N // P), 1), n_cores)
    if B_tot * P - N < 1:
        B_tot += n_cores          # guarantee >=1 pad node (the dummy target)
    N_pad = B_tot * P
    B_core = B_tot // n_cores
    NB = N_pad // P
    assert N_pad % G1 == 0

    nodes = np.concatenate([order, np.arange(N, N_pad, dtype=np.int64)])
    grid = nodes.reshape(B_tot, P)
    deg_pad = np.concatenate([deg, np.zeros(N_pad - N, np.int64)])

    eperm = np.argsort(dst, kind="stable")
    src_sorted = src[eperm]
    starts_full = np.concatenate([[0], np.cumsum(np.bincount(dst, minlength=N))])
    starts = np.zeros(N_pad, np.int64)
    starts[:N] = starts_full[:N]
    starts[N:] = starts_full[N]

    # K schedule per per-core block slot (same on every core -> same program)
    Ks = []
    for jb in range(B_core):
        m = 1
        for c in range(n_cores):
            m = max(m, int(deg_pad[grid[jb * n_cores + c]].max()))
        Ks.append(m)
    chunks1 = [_chunks_of(K, CK) for K in Ks]
    chunks2 = [_chunks_of(K, CK2) for K in Ks]
    S1 = sum(P * K for K in Ks)

    SH = B_core * P + P            # per-core z shard rows (+1 dummy block)
    agpos = np.zeros(N_pad, np.int64)
    own_all = []
    for c in range(n_cores):
        own = grid[c::n_cores].reshape(-1)
        own_all.append(own)
        agpos[own] = c * SH + np.arange(B_core * P)

    per_core = []
    for c in range(n_cores):
        own = grid[c::n_cores]                     # [B_core, P]
        own_flat = own.reshape(-1)
        rest = np.setdiff1d(np.arange(N_pad, dtype=np.int64), own_flat,
                            assume_unique=False)
        perm = np.concatenate([own_flat, rest])    # position -> node id
        rowpos = np.empty(N_pad, np.int64)
        rowpos[perm] = np.arange(N_pad)
        dummy_pos = rowpos[N]                      # node N is always a pad node

        idx1_parts, qoh1_parts = [], []
        idx2_parts, qoh2_parts = [], []
        for jb in range(B_core):
            nb = own[jb]
            dg = deg_pad[nb]
            st = starts[nb]
            K = Ks[jb]
            cols = np.arange(K)
            mask = cols[None, :] < dg[:, None]          # [P, K]
            pos = st[:, None] + np.where(mask, cols[None, :], 0)
            pos = np.minimum(pos, len(src_sorted) - 1)
            sv = src_sorted[pos]
            rp = np.where(mask, rowpos[sv], dummy_pos)  # [P, K] table positions
            ag = np.where(mask, agpos[sv], B_core * P)  # z-shard positions (c=0 dummy)
            ag = np.where(mask, ag, B_core * P)

            def build(blkmat, gpk, chunks):
                g = (blkmat // gpk).astype(np.int16)
                q = (blkmat % gpk).astype(np.int64)
                wr = []
                co = 0
                for ck in chunks:
                    wr.append(_wrap_idx(g[:, co:co + ck].T.ravel()))
                    co += ck
                wr = np.concatenate(wr, axis=1)         # [128, P*K/16]
                oh = np.zeros((P, blkmat.shape[1], gpk), np.float32)
                np.put_along_axis(oh, q[:, :, None], 1.0, axis=2)
                return wr.ravel(), oh.ravel()
            a, b = build(rp, G1, chunks1[jb])
            idx1_parts.append(a)
            qoh1_parts.append(b)
            a, b = build(ag, G2, chunks2[jb])
            idx2_parts.append(a)
            qoh2_parts.append(b)

        marks = np.zeros((P, NB), np.float32)
        pm = perm.reshape(NB, P).T                  # node id at [p, tile]
        marks[pm >= N] = -1e30

        per_core.append({
            "idx1": np.concatenate(idx1_parts),
            "qoh1": np.concatenate(qoh1_parts),
            "idx2": np.concatenate(idx2_parts),
            "qoh2": np.concatenate(qoh2_parts),
            "marks": marks,
            "_perm": perm,
        })

    # offsets into idx/qoh arrays per block
    def offsets(g):
        o_i, o_q = [], []
        oi = oq = 0
        for jb in range(B_core):
            o_i.append([oi])
            o_q.append([oq])
            oi += P * (P * Ks[jb] // 16)
            oq += P * Ks[jb] * g
        return o_i, o_q, oi, oq

    off1_i, off1_q, S1i, S1q = offsets(G1)
    off2_i, off2_q, S2i, S2q = offsets(G2)

    cfg = dict(n_cores=n_cores, B_core=B_core, NB=NB, N_pad=N_pad, SH=SH,
               Ks=Ks, chunks1=chunks1, chunks2=chunks2,
               off1_i=off1_i, off1_q=off1_q, off2_i=off2_i, off2_q=off2_q,
               S1i=S1i, S1q=S1q, S2i=S2i, S2q=S2q)
    return cfg, per_core, own_all


def _build(cfg):
    n_cores = cfg["n_cores"]
    B_core, NB, N_pad, SH = cfg["B_core"], cfg["NB"], cfg["N_pad"], cfg["SH"]
    Ks, chunks1, chunks2 = cfg["Ks"], cfg["chunks1"], cfg["chunks2"]
    off1_i, off1_q = cfg["off1_i"], cfg["off1_q"]
    off2_i, off2_q = cfg["off2_i"], cfg["off2_q"]
    T4 = N_pad // G1
    ZROWS = n_cores * SH // G2
    Kmax = max(Ks)

    nc = bacc.Bacc("TRN2", num_devices=n_cores, num_swdge_queues=4)

    xT_d = nc.dram_tensor("xT", [F_IN, N_pad], f32, kind="ExternalInput")
    w1_d = nc.dram_tensor("W1", [F_IN, HD], f32, kind="ExternalInput")
    aS_d = nc.dram_tensor("attS", [F_IN, HD], f32, kind="ExternalInput")
    aD_d = nc.dram_tensor("attD", [F_IN, HD], f32, kind="ExternalInput")
    b1_d = nc.dram_tensor("b1b", [P, HD], f32, kind="ExternalInput")
    w2_d = nc.dram_tensor("W2", [HD, C_OUT], f32, kind="ExternalInput")
    aS2_d = nc.dram_tensor("attS2", [HD, C_OUT], f32, kind="ExternalInput")
    aD2_d = nc.dram_tensor("attD2", [HD, C_OUT], f32, kind="ExternalInput")
    b2_d = nc.dram_tensor("b2b", [P, C_OUT], f32, kind="ExternalInput")
    mk_d = nc.dram_tensor("marks", [P, NB], f32, kind="ExternalInput")
    id_d = nc.dram_tensor("ident", [P, P], f32, kind="ExternalInput")
    idx1_d = nc.dram_tensor("idx1", [cfg["S1i"]], i16, kind="ExternalInput")
    qoh1_d = nc.dram_tensor("qoh1", [cfg["S1q"]], f32, kind="ExternalInput")
    idx2_d = nc.dram_tensor("idx2", [cfg["S2i"]], i16, kind="ExternalInput")
    qoh2_d = nc.dram_tensor("qoh2", [cfg["S2q"]], f32, kind="ExternalInput")

    table_d = nc.dram_tensor("table", [T4, G1 * RW], f32)
    zsh_d = nc.dram_tensor("zsh", [SH, ZW], f32)
    zag_space = "Shared" if n_cores > 4 else "Local"
    zag_d = nc.dram_tensor("zag", [ZROWS, G2 * ZW], f32, addr_space=zag_space)

    embo_d = nc.dram_tensor("emb_o", [B_core * P, HD], f32, kind="ExternalOutput")
    logp_d = nc.dram_tensor("logp_o", [B_core * P, C_OUT], f32, kind="ExternalOutput")

    with TileContext(nc) as tc:
        with (
            tc.tile_pool(name="const", bufs=1) as cp,
            tc.tile_pool(name="io", bufs=3) as iop,
            tc.tile_pool(name="gat", bufs=2) as gp,
            tc.tile_pool(name="gg", bufs=4) as ggp,
            tc.tile_pool(name="wk", bufs=3) as wk,
            tc.tile_pool(name="ps", bufs=2, space="PSUM") as ps,
        ):
            with tc.tile_critical():
                nc.gpsimd.load_library(library_config.mlp)
            qcnt = [0]

            # ---------------- setup consts ----------------
            W1_t = cp.tile([F_IN, HD], f32)
            nc.sync.dma_start(W1_t[:], w1_d[:, :])
            aS_t = cp.tile([F_IN, HD], f32)
            nc.sync.dma_start(aS_t[:], aS_d[:, :])
            aD_t = cp.tile([F_IN, HD], f32)
            nc.sync.dma_start(aD_t[:], aD_d[:, :])
            b1_t = cp.tile([P, HD], f32)
            nc.sync.dma_start(b1_t[:], b1_d[:, :])
            b2_t = cp.tile([P, C_OUT], f32)
            nc.sync.dma_start(b2_t[:], b2_d[:, :])
            mk_t = cp.tile([P, NB], f32)
            nc.sync.dma_start(mk_t[:], mk_d[:, :])
            ident = cp.tile([P, P], f32)
            nc.sync.dma_start(ident[:], id_d[:, :])

            Wx_t = cp.tile([F_IN, RW], f32)
            tmp0 = wk.tile([F_IN, HD], f32, tag="tmp0")
            p0 = tmp0[:].ap[0]
            tmp0_v = _mkap(tmp0[:], [p0, [DH, NH], [1, DH]])
            nc.vector.tensor_tensor(tmp0[:], W1_t[:], aS_t[:], AluOpType.mult)
            nc.vector.tensor_reduce(Wx_t[:, 64:72], tmp0_v, X_AX, AluOpType.add)
            nc.vector.tensor_tensor(tmp0[:], W1_t[:], aD_t[:], AluOpType.mult)
            nc.vector.tensor_reduce(Wx_t[:, 72:80], tmp0_v, X_AX, AluOpType.add)
            nc.vector.tensor_copy(Wx_t[:, 0:64], W1_t[:])

            W2_t = cp.tile([HD, C_OUT], f32)
            nc.sync.dma_start(W2_t[:], w2_d[:, :])
            aS2_t = cp.tile([HD, C_OUT], f32)
            nc.sync.dma_start(aS2_t[:], aS2_d[:, :])
            aD2_t = cp.tile([HD, C_OUT], f32)
            nc.sync.dma_start(aD2_t[:], aD2_d[:, :])
            W2e_t = cp.tile([HD, 9], f32)
            tmp2 = wk.tile([HD, C_OUT], f32, tag="tmp2")
            nc.vector.tensor_copy(W2e_t[:, 0:C_OUT], W2_t[:])
            nc.vector.tensor_tensor(tmp2[:], W2_t[:], aS2_t[:], AluOpType.mult)
            nc.vector.tensor_reduce(W2e_t[:, 7:8], tmp2[:], X_AX, AluOpType.add)
            nc.vector.tensor_tensor(tmp2[:], W2_t[:], aD2_t[:], AluOpType.mult)
            nc.vector.tensor_reduce(W2e_t[:, 8:9], tmp2[:], X_AX, AluOpType.add)

            adst_sb = cp.tile([P, B_core, NH], f32)   # own-node a_dst, block order
            zad_t = cp.tile([P, B_core], f32)         # own-node za_dst

            zdum = wk.tile([P, ZW], f32, tag="zdum")
            nc.vector.memset(zdum[:], 0.0)
            nc.vector.memset(zdum[:, 7:8], -1e30)
            nc.sync.dma_start(zsh_d[B_core * P:B_core * P + P, :], zdum[:])

            # ---------------- P0: build packed node table ----------------
            for i in range(NB):
                xt = iop.tile([P, P], f32, tag="xt")
                nc.sync.dma_start(
                    xt[:], _mkap(xT_d[0:1, 0:1], [[N_pad, P], [1, P]], i * P))
                hp = ps.tile([P, RW], f32, tag="hp")
                nc.tensor.matmul(hp[:], lhsT=xt[:], rhs=Wx_t[:], start=True, stop=True)
                row = iop.tile([P, RW], f32, tag="row")
                nc.vector.tensor_copy(row[:, 0:64], hp[:, 0:64])
                nc.vector.tensor_scalar(row[:, 64:80], hp[:, 64:80],
                                        mk_t[:, i:i + 1], None, AluOpType.add)
                if i < B_core:
                    nc.vector.tensor_copy(adst_sb[:, i, :], row[:, 72:80])
                nc.sync.dma_start(
                    _mkap(table_d[0:1, 0:1],
                          [[G1 * RW, P // G1], [RW, G1], [1, RW]],
                          i * P * RW),
                    row[:])

            # ---------------- P1: layer-1 edge pass ----------------
            for jb in range(B_core):
                na = wk.tile([P, HD], f32, tag="na")
                nc.vector.memset(na[:], 0.0)
                da = wk.tile([P, NH], f32, tag="da")
                nc.vector.memset(da[:], 0.0)
                ad = adst_sb[:, jb, :]
                ad_ap = ad.ap
                Kb = Ks[jb]
                nW = P * Kb // 16
                ixb = iop.tile([P, P * Kmax // 16], i16, tag="ixb")
                nc.sync.dma_start(
                    ixb[:, 0:nW],
                    _mkap(idx1_d[0:1], [[nW, P], [1, nW]], off1_i[jb][0]))
                qhb = iop.tile([P, Kmax, G1], f32, tag="qhb")
                nc.sync.dma_start(
                    qhb[:, 0:Kb, :],
                    _mkap(qoh1_d[0:1], [[Kb * G1, P], [1, Kb * G1]],
                          off1_q[jb][0]))
                co = 0
                for ci, ck in enumerate(chunks1[jb]):
                    n_idx = P * ck
                    ix = ixb[:, co * P // 16:co * P // 16 + n_idx // 16]
                    qh = qhb[:, co:co + CK] if ck == CK else qhb[:, co:co + ck]
                    G = ggp.tile([P, CK, G1 * RW], f32, tag="G")
                    nc.gpsimd.dma_gather(
                        out_ap=G[:, 0:ck, :], in_ap=table_d[:, :],
                        idxs_ap=ix,
                        num_idxs=n_idx, num_idxs_reg=n_idx,
                        elem_size=G1 * RW, single_packet=False,
                        queue_num=qcnt[0] % 4)
                    qcnt[0] += 1
                    # select quarter: hsel[p,k,:] = sum_q qoh * G[p,k,q,:]
                    T = gp.tile([P, CK, G1, RW], f32, tag="T")
                    Tp = T[:].ap[0]
                    T_v = _mkap(T[:], [Tp, [G1 * RW, ck], [RW, G1], [1, RW]])
                    G_v = _mkap(G[:], [G[:].ap[0], [G1 * RW, ck], [RW, G1], [1, RW]])
                    qh_v = _mkap(qh, [qh.ap[0], [G1, ck], [1, G1], [0, RW]])
                    nc.vector.tensor_tensor(T_v, G_v, qh_v, AluOpType.mult)
                    hs = gp.tile([P, CK, RW], f32, tag="hs")
                    hs_p = hs[:].ap[0]
                    T_r = _mkap(T[:], [Tp, [G1 * RW, ck], [1, RW], [RW, G1]])
                    nc.vector.tensor_reduce(
                        _mkap(hs[:], [hs_p, [RW, ck], [1, RW]]),
                        T_r, X_AX, AluOpType.add)
                    # e = a_src + a_dst ; lrelu ; exp
                    e = wk.tile([P, CK, NH], f32, tag="e")
                    ad_b = _mkap(ad, [ad_ap[0], [0, ck], ad_ap[1]])
                    nc.vector.tensor_tensor(
                        e[:, 0:ck, :],
                        _mkap(hs[:], [hs_p, [RW, ck], [1, NH]], 64),
                        ad_b, AluOpType.add)
                    nc.vector.scalar_tensor_tensor(e[:, 0:ck, :], e[:, 0:ck, :], 0.2,
                                                   e[:, 0:ck, :], AluOpType.mult,
                                                   AluOpType.max)
                    w = wk.tile([P, CK, NH], f32, tag="w")
                    nc.scalar.activation(w[:, 0:ck, :], e[:, 0:ck, :], Exp)
                    wp = w[:].ap[0]
                    dt_ = wk.tile([P, NH], f32, tag="dt")
                    w_perm = _mkap(w[:], [wp, [1, NH], [NH, ck]])
                    nc.vector.tensor_reduce(dt_[:], w_perm, X_AX, AluOpType.add)
                    nc.vector.tensor_add(da[:], da[:], dt_[:])
                    m = gp.tile([P, CK, HD], f32, tag="m")
                    mp = m[:].ap[0]
                    m_v = _mkap(m[:], [mp, [HD, ck], [DH, NH], [1, DH]])
                    h_v = _mkap(hs[:], [hs_p, [RW, ck], [DH, NH], [1, DH]])
                    w_v = _mkap(w[:], [wp, [NH, ck], [1, NH], [0, DH]])
                    nc.vector.tensor_tensor(m_v, h_v, w_v, AluOpType.mult)
                    nt_ = wk.tile([P, HD], f32, tag="nt")
                    m_perm = _mkap(m[:], [mp, [1, HD], [HD, ck]])
                    nc.vector.tensor_reduce(nt_[:], m_perm, X_AX, AluOpType.add)
                    nc.vector.tensor_add(na[:], na[:], nt_[:])
                    co += ck
                # epilogue: emb = num/den + b1 ; z rows
                nc.vector.tensor_scalar_add(da[:], da[:], 1e-16)
                rc = wk.tile([P, NH], f32, tag="rc")
                nc.vector.reciprocal(rc[:], da[:])
                emb = wk.tile([P, HD], f32, tag="emb")
                ep = emb[:].ap[0]
                emb_v = _mkap(emb[:], [ep, [DH, NH], [1, DH]])
                na_v = _mkap(na[:], [na[:].ap[0], [DH, NH], [1, DH]])
                rc_v = _mkap(rc[:], [rc[:].ap[0], [1, NH], [0, DH]])
                nc.vector.tensor_tensor(emb_v, na_v, rc_v, AluOpType.mult)
                nc.vector.tensor_add(emb[:], emb[:], b1_t[:])
                nc.sync.dma_start(embo_d[jb * P:(jb + 1) * P, :], emb[:])
                # elu -> z = elu(emb) @ [W2 | W2@attS2 | W2@attD2]
                t1 = wk.tile([P, HD], f32, tag="t1")
                nc.vector.tensor_scalar(t1[:], emb[:], 0.0, None, AluOpType.min)
                u = wk.tile([P, HD], f32, tag="u")
                nc.scalar.activation(u[:], t1[:], Exp)
                t2 = wk.tile([P, HD], f32, tag="t2")
                nc.vector.tensor_scalar(t2[:], emb[:], 0.0, None, AluOpType.max)
                el = wk.tile([P, HD], f32, tag="el")
                nc.vector.scalar_tensor_tensor(el[:], u[:], -1.0, t2[:],
                                               AluOpType.add, AluOpType.add)
                eT_p = ps.tile([HD, P], f32, tag="eTp")
                nc.tensor.transpose(eT_p[:], el[:], ident[:])
                eT = wk.tile([HD, P], f32, tag="eT")
                nc.vector.tensor_copy(eT[:], eT_p[:])
                zx = ps.tile([P, 9], f32, tag="zx")
                nc.tensor.matmul(zx[:], lhsT=eT[:], rhs=W2e_t[:], start=True, stop=True)
                z8 = wk.tile([P, ZW], f32, tag="z8")
                nc.vector.tensor_copy(z8[:], zx[:, 0:ZW])
                nc.vector.tensor_copy(zad_t[:, jb:jb + 1], zx[:, 8:9])
                nc.sync.dma_start(zsh_d[jb * P:(jb + 1) * P, :], z8[:])

            # ---------------- P2: exchange z rows ----------------
            nc.gpsimd.collective_compute(
                "AllGather", AluOpType.bypass,
                replica_groups=[list(range(n_cores))],
                ins=[zsh_d[:, :]], outs=[zag_d[:, :]])

            # ---------------- P3: layer-2 edge pass ----------------
            for jb in range(B_core):
                n2 = wk.tile([P, C_OUT], f32, tag="n2")
                nc.vector.memset(n2[:], 0.0)
                d2 = wk.tile([P, 1], f32, tag="d2")
                nc.vector.memset(d2[:], 0.0)
                za = zad_t[:, jb:jb + 1]
                Kb = Ks[jb]
                nW = P * Kb // 16
                ixb2 = iop.tile([P, P * Kmax // 16], i16, tag="ixb2")
                nc.sync.dma_start(
                    ixb2[:, 0:nW],
                    _mkap(idx2_d[0:1], [[nW, P], [1, nW]], off2_i[jb][0]))
                qhb2 = iop.tile([P, Kmax, G2], f32, tag="qhb2")
                nc.sync.dma_start(
                    qhb2[:, 0:Kb, :],
                    _mkap(qoh2_d[0:1], [[Kb * G2, P], [1, Kb * G2]],
                          off2_q[jb][0]))
                co = 0
                for ci, ck in enumerate(chunks2[jb]):
                    n_idx = P * ck
                    ix2 = ixb2[:, co * P // 16:co * P // 16 + n_idx // 16]
                    qh2 = qhb2[:, co:co + ck]
                    Gz = ggp.tile([P, CK2, G2 * ZW], f32, tag="Gz")
                    nc.gpsimd.dma_gather(
                        out_ap=Gz[:, 0:ck, :], in_ap=zag_d[:, :],
                        idxs_ap=ix2,
                        num_idxs=n_idx, num_idxs_reg=n_idx,
                        elem_size=G2 * ZW, single_packet=False,
                        queue_num=qcnt[0] % 4)
                    qcnt[0] += 1
                    T2 = gp.tile([P, CK2, G2, ZW], f32, tag="T2")
                    T2p = T2[:].ap[0]
                    T2_v = _mkap(T2[:], [T2p, [G2 * ZW, ck], [ZW, G2], [1, ZW]])
                    Gz_v = _mkap(Gz[:], [Gz[:].ap[0], [G2 * ZW, ck], [ZW, G2], [1, ZW]])
                    qh2_v = _mkap(qh2, [qh2.ap[0], [G2, ck], [1, G2], [0, ZW]])
                    nc.vector.tensor_tensor(T2_v, Gz_v, qh2_v, AluOpType.mult)
                    zs = gp.tile([P, CK2, ZW], f32, tag="zs")
                    zs_p = zs[:].ap[0]
                    T2_r = _mkap(T2[:], [T2p, [G2 * ZW, ck], [1, ZW], [ZW, G2]])
                    nc.vector.tensor_reduce(
                        _mkap(zs[:], [zs_p, [ZW, ck], [1, ZW]]),
                        T2_r, X_AX, AluOpType.add)
                    e2 = wk.tile([P, CK2], f32, tag="e2")
                    za_b = _mkap(za, [za.ap[0], [0, ck]])
                    nc.vector.tensor_tensor(
                        e2[:, 0:ck],
                        _mkap(zs[:], [zs_p, [ZW, ck]], 7),
                        za_b, AluOpType.add)
                    nc.vector.scalar_tensor_tensor(e2[:, 0:ck], e2[:, 0:ck], 0.2,
                                                   e2[:, 0:ck], AluOpType.mult,
                                                   AluOpType.max)
                    w2 = wk.tile([P, CK2], f32, tag="w2")
                    nc.scalar.activation(w2[:, 0:ck], e2[:, 0:ck], Exp)
                    d2t = wk.tile([P, 1], f32, tag="d2t")
                    nc.vector.tensor_reduce(d2t[:], w2[:, 0:ck], X_AX, AluOpType.add)
                    nc.vector.tensor_add(d2[:], d2[:], d2t[:])
                    m2 = gp.tile([P, CK2, C_OUT], f32, tag="m2")
                    m2p = m2[:].ap[0]
                    m2_v = _mkap(m2[:], [m2p, [C_OUT, ck], [1, C_OUT]])
                    zs_v = _mkap(zs[:], [zs_p, [ZW, ck], [1, C_OUT]])
                    w2_v = _mkap(w2[:], [w2[:].ap[0], [1, ck], [0, C_OUT]])
                    nc.vector.tensor_tensor(m2_v, zs_v, w2_v, AluOpType.mult)
                    n2t = wk.tile([P, C_OUT], f32, tag="n2t")
                    m2_perm = _mkap(m2[:], [m2p, [1, C_OUT], [C_OUT, ck]])
                    nc.vector.tensor_reduce(n2t[:], m2_perm, X_AX, AluOpType.add)
                    nc.vector.tensor_add(n2[:], n2[:], n2t[:])
                    co += ck
                # epilogue: logits + log_softmax
                nc.vector.tensor_scalar_add(d2[:], d2[:], 1e-16)
                rc2 = wk.tile([P, 1], f32, tag="rc2")
                nc.vector.reciprocal(rc2[:], d2[:])
                lg = wk.tile([P, C_OUT], f32, tag="lg")
                nc.vector.tensor_scalar(lg[:], n2[:], rc2[:, 0:1], None,
                                        AluOpType.mult)
                nc.vector.tensor_add(lg[:], lg[:], b2_t[:])
                mx = wk.tile([P, 1], f32, tag="mx")
                nc.vector.tensor_reduce(mx[:], lg[:], X_AX, AluOpType.max)
                s_ = wk.tile([P, C_OUT], f32, tag="s_")
                nc.vector.tensor_scalar(s_[:], lg[:], mx[:, 0:1], None,
                                        AluOpType.subtract)
                pp = wk.tile([P, C_OUT], f32, tag="pp")
                nc.scalar.activation(pp[:], s_[:], Exp)
                sm = wk.tile([P, 1], f32, tag="sm")
                nc.vector.tensor_reduce(sm[:], pp[:], X_AX, AluOpType.add)
                lt = wk.tile([P, 1], f32, tag="lt")
                nc.scalar.activation(lt[:], sm[:], Ln)
                lo = wk.tile([P, C_OUT], f32, tag="lo")
                nc.vector.tensor_scalar(lo[:], s_[:], lt[:, 0:1], None,
                                        AluOpType.subtract)
                nc.sync.dma_start(logp_d[jb * P:(jb + 1) * P, :], lo[:])
    nc.compile()
    return nc


def _make_inputs(inputs, cfg, per_core):
    x = np.asarray(inputs["x"], np.float32)
    N = x.shape[0]
    N_pad = cfg["N_pad"]
    shared = {
        "W1": np.asarray(inputs["W1"], np.float32),
        "attS": np.tile(np.asarray(inputs["att_src1"], np.float32).reshape(1, HD),
                        (F_IN, 1)),
        "attD": np.tile(np.asarray(inputs["att_dst1"], np.float32).reshape(1, HD),
                        (F_IN, 1)),
        "b1b": np.tile(np.asarray(inputs["b1"], np.float32).reshape(1, HD), (P, 1)),
        "W2": np.asarray(inputs["W2"], np.float32),
        "attS2": np.tile(np.asarray(inputs["att_src2"], np.float32).reshape(1, C_OUT),
                         (HD, 1)),
        "attD2": np.tile(np.asarray(inputs["att_dst2"], np.float32).reshape(1, C_OUT),
                         (HD, 1)),
        "b2b": np.tile(np.asarray(inputs["b2"], np.float32).reshape(1, C_OUT), (P, 1)),
        "ident": np.eye(P, dtype=np.float32),
    }
    xp = np.zeros((N_pad, F_IN), np.float32)
    xp[:N] = x
    in_maps = []
    for c in range(cfg["n_cores"]):
        m = dict(shared)
        perm = per_core[c]["_perm"]
        m["xT"] = np.ascontiguousarray(xp[perm].T)
        for k in ("idx1", "qoh1", "idx2", "qoh2", "marks"):
            m[k] = per_core[c][k]
        in_maps.append(m)
    return in_maps


def _assemble(cfg, own_all, N, results):
    emb = np.zeros((N, HD), np.float32)
    logp = np.zeros((N, C_OUT), np.float32)
    for c, res in enumerate(results):
        own = own_all[c]
        valid = own < N
        emb[own[valid]] = res["emb_o"][valid]
        logp[own[valid]] = res["logp_o"][valid]
    return emb, logp


def run_gat(inputs, n_cores=8, sim=False, trace=False):
    N = np.asarray(inputs["x"]).shape[0]
    cfg, per_core, own_all = _host_prep(inputs["edge_index"], N, n_cores)
    nc = _build(cfg)
    in_maps = _make_inputs(inputs, cfg, per_core)
    perf = None
    if sim:
        from concourse.bass_interp import CoreSim, MultiCoreSim
        if n_cores == 1:
            sims = [CoreSim(nc, require_finite=False, require_nnan=False)]
            for name, arr in in_maps[0].items():
                sims[0].tensor(name)[:] = arr
            sims[0].simulate()
        else:
            mcs = MultiCoreSim(nc, num_cores=n_cores,
                               require_finite=False, require_nnan=False)
            sims = list(mcs.cores.values())
            for c, s in enumerate(sims):
                for name, arr in in_maps[c].items():
                    s.tensor(name)[:] = arr
            mcs.simulate()
        results = [{"emb_o": s.tensor("emb_o"), "logp_o": s.tensor("logp_o")}
                   for s in sims]
    else:
        from concourse.bass_utils import run_bass_kernel_spmd
        r = run_bass_kernel_spmd(nc, in_maps, core_ids=list(range(n_cores)),
                                 trace=trace)
        results = r.results
        perf = r
    emb, logp = _assemble(cfg, own_all, N, results)
    return emb, logp, perf


def kernel(**inputs):
    emb, logp, _ = run_gat(inputs, n_cores=8)
    return emb, logp
